# revision 42
# baseline (speedup 1.0000x reference)
"""PixelMixBlock V2 Trainium2 kernel.

Full inputs in, full output out. Data-parallel over the batch: 32 samples
-> 8 NeuronCores x 4 samples. Per sample (on device, all matmuls bf16):

  q = Wq' x          [128, 784]   (Wq' = Wq * 128**-0.25, folds the 1/sqrt(d))
  k = Wq' x_g        [128, 784]   (x_g = x[index], gathered on host)
  v = Wv[:256] x_g   [1, 784]
  sT[c] = k[:,c*112:+112].T @ q            [112, 784] PSUM   (7 chunks)
  E[c]  = exp(sT[c])                       bf16 SBUF
  [num;den] += [v_chunk, 1].T @ E[c]       [2, 784] PSUM accumulate
  m = num/den + (Wv[256]*(1-lam) + bv)     [1, 784]
  O = A @ m.reshape(28,28) @ A.T           (bilinear x8 upsample as matmuls)
  y[1] = 1/(1+exp(-O)); y[0] = 1 - y[1]

exp/sigmoid both use the ACT "exp" table set (sigmoid via exp + DVE
reciprocal) so no activation-table reload ever happens.
"""

import numpy as np
import ml_dtypes

import concourse.bass as bass
import concourse.bacc as bacc
import concourse.mybir as mybir
import concourse.tile as tile
from concourse.bass_utils import run_bass_kernel_spmd

BF16 = mybir.dt.bfloat16
F32 = mybir.dt.float32
AF = mybir.ActivationFunctionType
ALU = mybir.AluOpType

N_CORES = 8
NS = 4            # samples per core
CIN = 256         # input channels
INTER = 128       # Wq output channels
HW = 784          # 28*28
CH = 28
OUT = 224         # 28 * scale_factor(8)
CW = 112          # q-chunk width (784 = 7*112)
NCH = 7           # number of q chunks
NH = 392          # matmul N half (784 = 2*392)
OH = 112          # output row chunk (224 = 2*112)


def _bilinear_matrix(n_in: int, n_out: int) -> np.ndarray:
    # Matches jax.image.resize(method='bilinear') for upscaling:
    # half-pixel sampling with edge clamp.
    scale = n_out / n_in
    A = np.zeros((n_out, n_in), np.float32)
    for i in range(n_out):
        src = (i + 0.5) / scale - 0.5
        j0 = int(np.floor(src))
        f = src - j0
        j0c = min(max(j0, 0), n_in - 1)
        j1c = min(max(j0 + 1, 0), n_in - 1)
        A[i, j0c] += 1.0 - f
        A[i, j1c] += f
    return A


def _build_program(dbg=False):
    nc = bacc.Bacc(
        "TRN2",
        target_bir_lowering=False,
        debug=False,
        enable_asserts=True,
        num_devices=N_CORES,
    )

    xq_d = nc.dram_tensor("xq", [NS, 128, 2 * HW], BF16, kind="ExternalInput").ap()
    xk_d = nc.dram_tensor("xk", [NS, 128, 2 * HW], BF16, kind="ExternalInput").ap()
    wqt_d = nc.dram_tensor("wqt", [128, 2 * INTER], BF16, kind="ExternalInput").ap()
    wvt_d = nc.dram_tensor("wvt", [128, 2], BF16, kind="ExternalInput").ap()
    at_d = nc.dram_tensor("at", [CH, OUT], BF16, kind="ExternalInput").ap()
    cvec_d = nc.dram_tensor("cvec", [CH, 1], F32, kind="ExternalInput").ap()
    y_d = nc.dram_tensor("y", [NS, 2, OUT, OUT], F32, kind="ExternalOutput").ap()
    if dbg:
        dq_d = nc.dram_tensor("dq", [NS, 128, HW], BF16, kind="ExternalOutput").ap()
        dk_d = nc.dram_tensor("dk", [NS, 128, HW], BF16, kind="ExternalOutput").ap()
        dv_d = nc.dram_tensor("dv", [NS, 1, HW], BF16, kind="ExternalOutput").ap()
        dw2_d = nc.dram_tensor("dw2", [NS, CW, 2 * NCH], BF16, kind="ExternalOutput").ap()
        de_d = nc.dram_tensor("de", [NS, CW, HW], BF16, kind="ExternalOutput").ap()
        dnd_d = nc.dram_tensor("dnd", [NS, 2, HW], F32, kind="ExternalOutput").ap()
        dmv_d = nc.dram_tensor("dmv", [NS, 1, HW], BF16, kind="ExternalOutput").ap()
        dmh_d = nc.dram_tensor("dmh", [NS, CH, CH], BF16, kind="ExternalOutput").ap()
        du_d = nc.dram_tensor("du", [NS, CH, OUT], BF16, kind="ExternalOutput").ap()

    with tile.TileContext(nc) as tc:
        with (
            tc.tile_pool(name="const", bufs=1) as cpool,
            tc.tile_pool(name="xin", bufs=NS) as xpool,
            tc.tile_pool(name="qk", bufs=2) as qkpool,
            tc.tile_pool(name="ew", bufs=6) as epool,
            tc.tile_pool(name="mk", bufs=2) as mkpool,
            tc.tile_pool(name="sg", bufs=3) as sgpool,
            tc.tile_pool(name="pqk", bufs=1, space="PSUM") as pqk,
            tc.tile_pool(name="pst", bufs=2, space="PSUM") as pst,
            tc.tile_pool(name="pnd", bufs=1, space="PSUM") as pnd,
        ):
            # --- input DMAs up front; sample-0 + weights first so the
            # first matmul starts as early as possible ---
            wqt_sb = cpool.tile([128, 2 * INTER], BF16, tag="wqt")
            wvt_sb = cpool.tile([128, 2], BF16, tag="wvt")
            at_sb = cpool.tile([CH, OUT], BF16, tag="at")
            cvec_sb = cpool.tile([CH, 1], F32, tag="cvec")
            xq_sbs, xk_sbs = [], []
            for s in range(NS):
                xq_sb = xpool.tile([128, 2 * HW], BF16, tag="xq")
                xk_sb = xpool.tile([128, 2 * HW], BF16, tag="xk")
                xq_sbs.append(xq_sb)
                xk_sbs.append(xk_sb)
            nc.sync.dma_start(wqt_sb[:], wqt_d[:])
            # sample-0 inputs split by channel-chunk so the first matmuls can
            # start before the full tensors land
            nc.sync.dma_start(xq_sbs[0][:, 0:HW], xq_d[0][:, 0:HW])
            nc.sync.dma_start(xk_sbs[0][:, 0:HW], xk_d[0][:, 0:HW])
            nc.sync.dma_start(xq_sbs[0][:, HW:2 * HW], xq_d[0][:, HW:2 * HW])
            nc.sync.dma_start(xk_sbs[0][:, HW:2 * HW], xk_d[0][:, HW:2 * HW])
            nc.sync.dma_start(wvt_sb[:], wvt_d[:])
            for s in range(1, NS):
                nc.sync.dma_start(xq_sbs[s][:], xq_d[s])
                nc.sync.dma_start(xk_sbs[s][:], xk_d[s])
            nc.sync.dma_start(at_sb[:], at_d[:])
            nc.sync.dma_start(cvec_sb[:], cvec_d[:])

            # two persistent w2 tiles: [112, 33*7]; per chunk c the nd lhsT is
            # cols [33c, 33c+33): col 33c = v, cols +1..+31 = 0, col +32 = 1.
            # den then lands on PSUM partition 32 (32-aligned for DVE reads).
            W2W = 33
            w2s = []
            for i in range(2):
                w2t = cpool.tile([CW, W2W * NCH], BF16, tag=f"w2_{i}")
                nc.vector.memset(w2t[:], 0.0)
                nc.vector.memset(w2t[:, 32:W2W * NCH:W2W], 1.0)
                w2s.append(w2t)

            def emit_head(s):
                """q/k/v matmuls + drains + w2 scatter for sample s."""
                xq_sb, xk_sb = xq_sbs[s], xk_sbs[s]
                q_sb = qkpool.tile([128, HW], BF16, tag="q")
                k_sb = qkpool.tile([128, HW], BF16, tag="k")
                for (src, dst) in ((xq_sb, q_sb), (xk_sb, k_sb)):
                    # [128, 2, 512] so each 392-wide half is PSUM-bank aligned
                    qps = pqk.tile([128, 2, 512], F32, tag="qk")
                    for c in range(2):
                        for h in range(2):
                            nc.tensor.matmul(
                                qps[:, h, 0:NH],
                                wqt_sb[:, c * INTER:(c + 1) * INTER],
                                src[:, c * HW + h * NH: c * HW + (h + 1) * NH],
                                start=(c == 0),
                                stop=(c == 1),
                            )
                    nc.vector.tensor_copy(
                        dst[:].rearrange("p (h f) -> p h f", h=2),
                        qps[:, :, 0:NH],
                    )

                # vT computed directly on PE: for q-chunk c,
                # vT[qq] = sum_i x_g[i, c*112+qq] * Wv[i]  (lhsT = x_g slice)
                vtp = pqk.tile([CW, NCH], F32, tag="qk")
                for c in range(NCH):
                    for ci in range(2):
                        nc.tensor.matmul(
                            vtp[:, c:c + 1],
                            xk_sb[:, ci * HW + c * CW: ci * HW + (c + 1) * CW],
                            wvt_sb[:, ci:ci + 1],
                            start=(ci == 0),
                            stop=(ci == 1),
                            skip_group_check=True,
                        )
                w2 = w2s[s % 2]
                nc.vector.tensor_copy(w2[:, 0:W2W * NCH:W2W], vtp[:])
                if dbg:
                    nc.sync.dma_start(dq_d[s], q_sb[:])
                    nc.sync.dma_start(dk_d[s], k_sb[:])
                return q_sb, k_sb, w2

            def emit_attention(s, q_sb, k_sb, w2):
                """sT -> exp -> num/den accumulate; returns PSUM nd tile."""
                ndps = pnd.tile([W2W, 2, 512], F32, tag="nd")

                def emit_st(c):
                    stps = pst.tile([CW, 2, 512], F32, tag="st")
                    for h in range(2):
                        nc.tensor.matmul(
                            stps[:, h, 0:NH],
                            k_sb[:, c * CW:(c + 1) * CW],
                            q_sb[:, h * NH:(h + 1) * NH],
                            start=True,
                            stop=True,
                        )
                    return stps

                # PE stream: sT(0), sT(1), nd(0), sT(2), nd(1), ... so the
                # next chunk's sT never queues behind an nd that waits on exp
                stps = emit_st(0)
                for c in range(NCH):
                    et = epool.tile([CW, HW], BF16, tag="e")
                    nc.scalar.activation(
                        et[:].rearrange("p (h f) -> p h f", h=2),
                        stps[:, :, 0:NH],
                        AF.Exp,
                    )
                    if dbg and c == 0:
                        nc.sync.dma_start(de_d[s], et[:])
                    if c + 1 < NCH:
                        stps = emit_st(c + 1)
                    for h in range(2):
                        nc.tensor.matmul(
                            ndps[:, h, 0:NH],
                            w2[:, W2W * c:W2W * (c + 1)],
                            et[:, h * NH:(h + 1) * NH],
                            start=(c == 0),
                            stop=(c == NCH - 1),
                            skip_group_check=True,
                        )
                return ndps

            def emit_tail(s, ndps):
                """mask reshape + divide, bilinear upsample, sigmoid, store."""
                nd_sb = mkpool.tile([W2W, HW], F32, tag="nd_sb")
                nc.vector.tensor_copy(
                    nd_sb[:].rearrange("p (h f) -> p h f", h=2),
                    ndps[:, :, 0:NH],
                )
                mhn = mkpool.tile([CH, CH], F32, tag="mhn")
                mhd = mkpool.tile([CH, CH], F32, tag="mhd")
                nc.sync.dma_start(
                    mhn[:], nd_sb[0:1, :].rearrange("p (a b) -> p a b", a=CH))
                nc.gpsimd.dma_start(
                    mhd[:], nd_sb[32:33, :].rearrange("p (a b) -> p a b", a=CH))
                rcd = mkpool.tile([CH, CH], F32, tag="rcd")
                nc.vector.reciprocal(rcd[:], mhd[:])
                mtm = mkpool.tile([CH, CH], F32, tag="mtm")
                nc.vector.tensor_tensor(mtm[:], mhn[:], rcd[:], ALU.mult)
                mh = mkpool.tile([CH, CH], BF16, tag="mh")
                nc.vector.tensor_scalar(
                    mh[:], mtm[:], cvec_sb[:, 0:1], None, ALU.add
                )
                if dbg:
                    nc.sync.dma_start(dnd_d[s, 0:1], nd_sb[0:1, :])
                    nc.sync.dma_start(dnd_d[s, 1:2], nd_sb[32:33, :])
                    nc.sync.dma_start(dmh_d[s], mh[:])

                ups = pqk.tile([CH, OUT], F32, tag="qk")
                nc.tensor.matmul(ups[:], mh[:], at_sb[:], start=True, stop=True)
                u_sb = mkpool.tile([CH, OUT], BF16, tag="u")
                nc.vector.tensor_copy(u_sb[:], ups[:])
                if dbg:
                    nc.sync.dma_start(du_d[s], u_sb[:])

                # both 112-row output chunks in one PSUM tile (bank-aligned
                # halves), one sigmoid-exp over both, fewer DVE ops
                ops = pqk.tile([OH, 2, 512], F32, tag="qk")
                for j in range(2):
                    nc.tensor.matmul(
                        ops[:, j, 0:OUT], u_sb[:, j * OH:(j + 1) * OH],
                        at_sb[:], start=True, stop=True,
                    )
                es = sgpool.tile([OH, 2, OUT], F32, tag="es")
                nc.scalar.activation(es[:], ops[:, :, 0:OUT], AF.Exp, scale=-1.0)
                t1 = sgpool.tile([OH, 2, OUT], F32, tag="t1")
                nc.vector.tensor_scalar(t1[:], es[:], 1.0, None, ALU.add)
                # sb4 layout: [112, (j, ch), 224]
                sb4 = sgpool.tile([OH, 4, OUT], F32, tag="sb4")
                nc.vector.reciprocal(
                    sb4[:].rearrange("p (j c) f -> p j c f", j=2)[:, :, 1, :],
                    t1[:],
                )
                nc.vector.tensor_scalar(
                    sb4[:].rearrange("p (j c) f -> p j c f", j=2)[:, :, 0, :],
                    sb4[:].rearrange("p (j c) f -> p j c f", j=2)[:, :, 1, :],
                    -1.0, 1.0, ALU.mult, ALU.add,
                )
                for j in range(2):
                    nc.sync.dma_start(
                        y_d[s, :, j * OH:(j + 1) * OH, :]
                        .rearrange("c p f -> p c f"),
                        sb4[:, 2 * j:2 * j + 2, :],
                    )

            # software pipeline: emit head(s+1) before tail(s) so the next
            # sample's drains are not queued behind this sample's tail
            ctx = emit_head(0)
            for s in range(NS):
                ndps = emit_attention(s, *ctx)
                if s + 1 < NS:
                    ctx = emit_head(s + 1)
                emit_tail(s, ndps)

    nc.compile()
    return nc


_NC_CACHE = {}


def _get_program(dbg=False):
    if dbg not in _NC_CACHE:
        _NC_CACHE[dbg] = _build_program(dbg)
    return _NC_CACHE[dbg]


def kernel(x, lam, index, scale_factor, Wq, bq, Wv, bv):
    x = np.asarray(x, dtype=np.float32)
    lam = np.asarray(lam, dtype=np.float32)
    index = np.asarray(index).astype(np.int64)
    Wq = np.asarray(Wq, dtype=np.float32)
    Wv = np.asarray(Wv, dtype=np.float32)
    bv = np.asarray(bv, dtype=np.float32)

    n, C, h, w = x.shape
    bf = ml_dtypes.bfloat16

    xr = x.reshape(n, C, h * w)
    xg = xr[index]

    s4 = np.float32(INTER) ** np.float32(-0.25)
    WqT = np.ascontiguousarray((Wq * s4).T).astype(bf)          # [256, 128]
    wqt = np.ascontiguousarray(
        WqT.reshape(2, 128, INTER).transpose(1, 0, 2).reshape(128, 2 * INTER))
    wvt = np.ascontiguousarray(Wv[0, :C].astype(bf).reshape(2, 128).T)
    const = np.float32(Wv[0, C] * (1.0 - lam[0]) + bv[0])
    cvec = np.full((CH, 1), const, np.float32)
    A = _bilinear_matrix(CH, OUT)
    at = np.ascontiguousarray(A.T).astype(bf)                   # [28, 224]

    xq_all = np.ascontiguousarray(
        xr.reshape(n, 2, 128, h * w).transpose(0, 2, 1, 3)
        .reshape(n, 128, 2 * h * w)).astype(bf)
    xk_all = np.ascontiguousarray(
        xg.reshape(n, 2, 128, h * w).transpose(0, 2, 1, 3)
        .reshape(n, 128, 2 * h * w)).astype(bf)

    import os
    dbg = bool(os.environ.get("DBGTAPS"))
    nc = _get_program(dbg)
    core_ids = list(range(N_CORES))
    in_maps = []
    for i in core_ids:
        sl = slice(i * NS, (i + 1) * NS)
        in_maps.append({
            "xq": np.ascontiguousarray(xq_all[sl]),
            "xk": np.ascontiguousarray(xk_all[sl]),
            "wqt": wqt,
            "wvt": wvt,
            "at": at,
            "cvec": cvec,
        })

    res = run_bass_kernel_spmd(nc, in_maps, core_ids)
    global LAST_RESULTS
    LAST_RESULTS = res
    out = np.concatenate([r["y"] for r in res.results], axis=0)
    return out.astype(np.float32)


LAST_RESULTS = None


if __name__ == "__main__":
    # smoke test with random data
    rng = np.random.default_rng(0)
    inputs = {
        "x": rng.standard_normal((32, 256, 28, 28), dtype=np.float32),
        "lam": rng.random((1,), dtype=np.float32),
        "index": rng.integers(0, 32, (32,)),
        "scale_factor": 8,
        "Wq": (rng.standard_normal((128, 256)) * 0.01).astype(np.float32),
        "bq": np.zeros((128,), np.float32),
        "Wv": (rng.standard_normal((1, 257)) * 0.01).astype(np.float32),
        "bv": np.zeros((1,), np.float32),
    }
    y = kernel(**inputs)
    print("out", y.shape, y.dtype, float(y.min()), float(y.max()))


# revision 44
# speedup vs baseline: 1.0623x; 1.0623x over previous
"""PixelMixBlock V2 Trainium2 kernel.

Full inputs in, full output out. Data-parallel over the batch: 32 samples
-> 8 NeuronCores x 4 samples. Per sample (on device, all matmuls bf16):

  q = Wq' x          [128, 784]   (Wq' = Wq * 128**-0.25, folds the 1/sqrt(d))
  k = Wq' x_g        [128, 784]   (x_g = x[index], gathered on host)
  v = Wv[:256] x_g   [1, 784]
  sT[c] = k[:,c*112:+112].T @ q            [112, 784] PSUM   (7 chunks)
  E[c]  = exp(sT[c])                       bf16 SBUF
  [num;den] += [v_chunk, 1].T @ E[c]       [2, 784] PSUM accumulate
  m = num/den + (Wv[256]*(1-lam) + bv)     [1, 784]
  O = A @ m.reshape(28,28) @ A.T           (bilinear x8 upsample as matmuls)
  y[1] = 1/(1+exp(-O)); y[0] = 1 - y[1]

exp/sigmoid both use the ACT "exp" table set (sigmoid via exp + DVE
reciprocal) so no activation-table reload ever happens.
"""

import numpy as np
import ml_dtypes

import concourse.bass as bass
import concourse.bacc as bacc
import concourse.mybir as mybir
import concourse.tile as tile
from concourse.bass_utils import run_bass_kernel_spmd

BF16 = mybir.dt.bfloat16
F32 = mybir.dt.float32
AF = mybir.ActivationFunctionType
ALU = mybir.AluOpType

N_CORES = 8
NS = 4            # samples per core
CIN = 256         # input channels
INTER = 128       # Wq output channels
HW = 784          # 28*28
CH = 28
OUT = 224         # 28 * scale_factor(8)
CW = 112          # q-chunk width (784 = 7*112)
NCH = 7           # number of q chunks
NH = 392          # matmul N half (784 = 2*392)
OH = 112          # output row chunk (224 = 2*112)
W2W = 33          # nd lhsT width (v col + 31 zeros + ones col)


def _bilinear_matrix(n_in: int, n_out: int) -> np.ndarray:
    # Matches jax.image.resize(method='bilinear') for upscaling:
    # half-pixel sampling with edge clamp.
    scale = n_out / n_in
    A = np.zeros((n_out, n_in), np.float32)
    for i in range(n_out):
        src = (i + 0.5) / scale - 0.5
        j0 = int(np.floor(src))
        f = src - j0
        j0c = min(max(j0, 0), n_in - 1)
        j1c = min(max(j0 + 1, 0), n_in - 1)
        A[i, j0c] += 1.0 - f
        A[i, j1c] += f
    return A


def _build_program(dbg=False):
    nc = bacc.Bacc(
        "TRN2",
        target_bir_lowering=False,
        debug=False,
        enable_asserts=True,
        num_devices=N_CORES,
    )

    xq_d = nc.dram_tensor("xq", [NS, 128, 2 * HW], BF16, kind="ExternalInput").ap()
    xk_d = nc.dram_tensor("xk", [NS, 128, 2 * HW], BF16, kind="ExternalInput").ap()
    wqt_d = nc.dram_tensor("wqt", [128, 2 * INTER], BF16, kind="ExternalInput").ap()
    wvt_d = nc.dram_tensor("wvt", [128, 2], BF16, kind="ExternalInput").ap()
    at_d = nc.dram_tensor("at", [CH, OUT], BF16, kind="ExternalInput").ap()
    cvec_d = nc.dram_tensor("cvec", [CH, 1], F32, kind="ExternalInput").ap()
    y_d = nc.dram_tensor("y", [NS, 2, OUT, OUT], F32, kind="ExternalOutput").ap()
    if dbg:
        dq_d = nc.dram_tensor("dq", [NS, 128, HW], BF16, kind="ExternalOutput").ap()
        dk_d = nc.dram_tensor("dk", [NS, 128, HW], BF16, kind="ExternalOutput").ap()
        dv_d = nc.dram_tensor("dv", [NS, 1, HW], BF16, kind="ExternalOutput").ap()
        dw2_d = nc.dram_tensor("dw2", [NS, CW, 2 * NCH], BF16, kind="ExternalOutput").ap()
        de_d = nc.dram_tensor("de", [NS, CW, HW], BF16, kind="ExternalOutput").ap()
        dnd_d = nc.dram_tensor("dnd", [NS, 2, HW], F32, kind="ExternalOutput").ap()
        dmv_d = nc.dram_tensor("dmv", [NS, 1, HW], BF16, kind="ExternalOutput").ap()
        dmh_d = nc.dram_tensor("dmh", [NS, CH, CH], BF16, kind="ExternalOutput").ap()
        du_d = nc.dram_tensor("du", [NS, CH, OUT], BF16, kind="ExternalOutput").ap()

    with tile.TileContext(nc) as tc:
        with (
            tc.tile_pool(name="const", bufs=1) as cpool,
            tc.tile_pool(name="xin", bufs=NS) as xpool,
            tc.tile_pool(name="qk", bufs=2) as qkpool,
            tc.tile_pool(name="ew", bufs=6) as epool,
            tc.tile_pool(name="mk", bufs=2) as mkpool,
            tc.tile_pool(name="sg", bufs=3) as sgpool,
            tc.tile_pool(name="pqk", bufs=1, space="PSUM") as pqk,
            tc.tile_pool(name="pst", bufs=2, space="PSUM") as pst,
            tc.tile_pool(name="pnd", bufs=1, space="PSUM") as pnd,
        ):
            # --- input DMAs up front; sample-0 + weights first so the
            # first matmul starts as early as possible ---
            wqt_sb = cpool.tile([128, 2 * INTER], BF16, tag="wqt")
            wvt_sb = cpool.tile([128, 2], BF16, tag="wvt")
            at_sb = cpool.tile([CH, OUT], BF16, tag="at")
            cvec_sb = cpool.tile([CH, 1], F32, tag="cvec")
            one33 = cpool.tile([W2W, 1], F32, tag="one33")
            xq_sbs, xk_sbs = [], []
            for s in range(NS):
                xq_sb = xpool.tile([128, 2 * HW], BF16, tag="xq")
                xk_sb = xpool.tile([128, 2 * HW], BF16, tag="xk")
                xq_sbs.append(xq_sb)
                xk_sbs.append(xk_sb)
            nc.sync.dma_start(wqt_sb[:], wqt_d[:])
            # sample-0 inputs split by channel-chunk so the first matmuls can
            # start before the full tensors land
            nc.sync.dma_start(xq_sbs[0][:, 0:HW], xq_d[0][:, 0:HW])
            nc.sync.dma_start(xk_sbs[0][:, 0:HW], xk_d[0][:, 0:HW])
            nc.sync.dma_start(xq_sbs[0][:, HW:2 * HW], xq_d[0][:, HW:2 * HW])
            nc.sync.dma_start(xk_sbs[0][:, HW:2 * HW], xk_d[0][:, HW:2 * HW])
            nc.sync.dma_start(wvt_sb[:], wvt_d[:])
            for s in range(1, NS):
                nc.sync.dma_start(xq_sbs[s][:], xq_d[s])
                nc.sync.dma_start(xk_sbs[s][:], xk_d[s])
            nc.sync.dma_start(at_sb[:], at_d[:])
            nc.sync.dma_start(cvec_sb[:], cvec_d[:])

            # two persistent w2 tiles: [112, 33*7]; per chunk c the nd lhsT is
            # cols [33c, 33c+33): col 33c = v, cols +1..+31 = 0, col +32 = 1.
            # den then lands on PSUM partition 32 (32-aligned for DVE reads).
            nc.vector.memset(one33[:], 1.0)
            w2s = []
            for i in range(2):
                w2t = cpool.tile([CW, W2W * NCH], BF16, tag=f"w2_{i}")
                nc.vector.memset(w2t[:], 0.0)
                nc.vector.memset(w2t[:, 32:W2W * NCH:W2W], 1.0)
                w2s.append(w2t)

            def emit_head(s):
                """q/k/v matmuls + drains + w2 scatter for sample s."""
                xq_sb, xk_sb = xq_sbs[s], xk_sbs[s]
                q_sb = qkpool.tile([128, HW], BF16, tag="q")
                k_sb = qkpool.tile([128, HW], BF16, tag="k")
                for (src, dst) in ((xq_sb, q_sb), (xk_sb, k_sb)):
                    # [128, 2, 512] so each 392-wide half is PSUM-bank aligned
                    qps = pqk.tile([128, 2, 512], F32, tag="qk")
                    for c in range(2):
                        for h in range(2):
                            nc.tensor.matmul(
                                qps[:, h, 0:NH],
                                wqt_sb[:, c * INTER:(c + 1) * INTER],
                                src[:, c * HW + h * NH: c * HW + (h + 1) * NH],
                                start=(c == 0),
                                stop=(c == 1),
                            )
                    nc.vector.tensor_copy(
                        dst[:].rearrange("p (h f) -> p h f", h=2),
                        qps[:, :, 0:NH],
                    )

                # vT computed directly on PE: for q-chunk c,
                # vT[qq] = sum_i x_g[i, c*112+qq] * Wv[i]  (lhsT = x_g slice)
                vtp = pqk.tile([CW, NCH], F32, tag="qk")
                for c in range(NCH):
                    for ci in range(2):
                        nc.tensor.matmul(
                            vtp[:, c:c + 1],
                            xk_sb[:, ci * HW + c * CW: ci * HW + (c + 1) * CW],
                            wvt_sb[:, ci:ci + 1],
                            start=(ci == 0),
                            stop=(ci == 1),
                            skip_group_check=True,
                        )
                w2 = w2s[s % 2]
                nc.vector.tensor_copy(w2[:, 0:W2W * NCH:W2W], vtp[:])
                if dbg:
                    nc.sync.dma_start(dq_d[s], q_sb[:])
                    nc.sync.dma_start(dk_d[s], k_sb[:])
                return q_sb, k_sb, w2

            def emit_attention(s, q_sb, k_sb, w2):
                """sT -> exp -> num/den accumulate; returns PSUM nd tile."""
                ndps = pnd.tile([W2W, 2, 512], F32, tag="nd")

                def emit_st(c):
                    stps = pst.tile([CW, 2, 512], F32, tag="st")
                    for h in range(2):
                        nc.tensor.matmul(
                            stps[:, h, 0:NH],
                            k_sb[:, c * CW:(c + 1) * CW],
                            q_sb[:, h * NH:(h + 1) * NH],
                            start=True,
                            stop=True,
                        )
                    return stps

                # PE stream: sT(0), sT(1), nd(0), sT(2), nd(1), ... so the
                # next chunk's sT never queues behind an nd that waits on exp
                stps = emit_st(0)
                for c in range(NCH):
                    et = epool.tile([CW, HW], BF16, tag="e")
                    nc.scalar.activation(
                        et[:].rearrange("p (h f) -> p h f", h=2),
                        stps[:, :, 0:NH],
                        AF.Exp,
                    )
                    if dbg and c == 0:
                        nc.sync.dma_start(de_d[s], et[:])
                    if c + 1 < NCH:
                        stps = emit_st(c + 1)
                    for h in range(2):
                        nc.tensor.matmul(
                            ndps[:, h, 0:NH],
                            w2[:, W2W * c:W2W * (c + 1)],
                            et[:, h * NH:(h + 1) * NH],
                            start=(c == 0),
                            stop=(c == NCH - 1),
                            skip_group_check=True,
                        )
                return ndps

            def emit_tail(s, ndps):
                """mask reshape + divide, bilinear upsample, sigmoid, store."""
                nd_sb = mkpool.tile([W2W, HW], F32, tag="nd_sb")
                nc.vector.tensor_copy(
                    nd_sb[:].rearrange("p (h f) -> p h f", h=2),
                    ndps[:, :, 0:NH],
                )
                # reshape num/den rows to [28, 28] transposed layout with
                # 28 tiny PE transposes each (PE is idle here; avoids the
                # ~2.7us DMA latency in the tail chain)
                mtn_ps = pqk.tile([CH, CH], F32, tag="qk")
                for h in range(CH):
                    nc.tensor.transpose(
                        mtn_ps[:, h:h + 1],
                        nd_sb[0:1, CH * h:CH * (h + 1)],
                        one33[0:1, 0:1],
                    )
                mtd_ps = pnd.tile([CH, CH], F32, tag="nd")
                for h in range(CH):
                    nc.tensor.transpose(
                        mtd_ps[:, h:h + 1],
                        nd_sb[32:33, CH * h:CH * (h + 1)],
                        one33[32:33, 0:1],
                    )
                mhn = mkpool.tile([CH, CH], F32, tag="mhn")
                nc.vector.tensor_copy(mhn[:], mtn_ps[:])
                mhd = mkpool.tile([CH, CH], F32, tag="mhd")
                nc.vector.tensor_copy(mhd[:], mtd_ps[:])
                rcd = mkpool.tile([CH, CH], F32, tag="rcd")
                nc.vector.reciprocal(rcd[:], mhd[:])
                mtm = mkpool.tile([CH, CH], F32, tag="mtm")
                nc.vector.tensor_tensor(mtm[:], mhn[:], rcd[:], ALU.mult)
                # mh here is M^T: [w on partitions, h free]
                mh = mkpool.tile([CH, CH], BF16, tag="mh")
                nc.vector.tensor_scalar(
                    mh[:], mtm[:], cvec_sb[:, 0:1], None, ALU.add
                )
                if dbg:
                    nc.sync.dma_start(dnd_d[s, 0:1], nd_sb[0:1, :])
                    nc.sync.dma_start(dnd_d[s, 1:2], nd_sb[32:33, :])
                    nc.sync.dma_start(dmh_d[s], mh[:])

                ups = pqk.tile([CH, OUT], F32, tag="qk")
                nc.tensor.matmul(ups[:], mh[:], at_sb[:], start=True, stop=True)
                u_sb = mkpool.tile([CH, OUT], BF16, tag="u")
                nc.vector.tensor_copy(u_sb[:], ups[:])
                if dbg:
                    nc.sync.dma_start(du_d[s], u_sb[:])

                # both 112-row output chunks in one PSUM tile (bank-aligned
                # halves), one sigmoid-exp over both, fewer DVE ops
                ops = pqk.tile([OH, 2, 512], F32, tag="qk")
                for j in range(2):
                    nc.tensor.matmul(
                        ops[:, j, 0:OUT], at_sb[:, j * OH:(j + 1) * OH],
                        u_sb[:], start=True, stop=True,
                    )
                es = sgpool.tile([OH, 2, OUT], F32, tag="es")
                nc.scalar.activation(es[:], ops[:, :, 0:OUT], AF.Exp, scale=-1.0)
                t1 = sgpool.tile([OH, 2, OUT], F32, tag="t1")
                nc.vector.tensor_scalar(t1[:], es[:], 1.0, None, ALU.add)
                # sb4 layout: [112, (j, ch), 224]
                sb4 = sgpool.tile([OH, 4, OUT], F32, tag="sb4")
                nc.vector.reciprocal(
                    sb4[:].rearrange("p (j c) f -> p j c f", j=2)[:, :, 1, :],
                    t1[:],
                )
                nc.vector.tensor_scalar(
                    sb4[:].rearrange("p (j c) f -> p j c f", j=2)[:, :, 0, :],
                    sb4[:].rearrange("p (j c) f -> p j c f", j=2)[:, :, 1, :],
                    -1.0, 1.0, ALU.mult, ALU.add,
                )
                for j in range(2):
                    nc.sync.dma_start(
                        y_d[s, :, j * OH:(j + 1) * OH, :]
                        .rearrange("c p f -> p c f"),
                        sb4[:, 2 * j:2 * j + 2, :],
                    )

            # software pipeline: emit head(s+1) before tail(s) so the next
            # sample's drains are not queued behind this sample's tail
            ctx = emit_head(0)
            for s in range(NS):
                ndps = emit_attention(s, *ctx)
                if s + 1 < NS:
                    ctx = emit_head(s + 1)
                emit_tail(s, ndps)

    nc.compile()
    return nc


_NC_CACHE = {}


def _get_program(dbg=False):
    if dbg not in _NC_CACHE:
        _NC_CACHE[dbg] = _build_program(dbg)
    return _NC_CACHE[dbg]


def kernel(x, lam, index, scale_factor, Wq, bq, Wv, bv):
    x = np.asarray(x, dtype=np.float32)
    lam = np.asarray(lam, dtype=np.float32)
    index = np.asarray(index).astype(np.int64)
    Wq = np.asarray(Wq, dtype=np.float32)
    Wv = np.asarray(Wv, dtype=np.float32)
    bv = np.asarray(bv, dtype=np.float32)

    n, C, h, w = x.shape
    bf = ml_dtypes.bfloat16

    xr = x.reshape(n, C, h * w)
    xg = xr[index]

    s4 = np.float32(INTER) ** np.float32(-0.25)
    WqT = np.ascontiguousarray((Wq * s4).T).astype(bf)          # [256, 128]
    wqt = np.ascontiguousarray(
        WqT.reshape(2, 128, INTER).transpose(1, 0, 2).reshape(128, 2 * INTER))
    wvt = np.ascontiguousarray(Wv[0, :C].astype(bf).reshape(2, 128).T)
    const = np.float32(Wv[0, C] * (1.0 - lam[0]) + bv[0])
    cvec = np.full((CH, 1), const, np.float32)
    A = _bilinear_matrix(CH, OUT)
    at = np.ascontiguousarray(A.T).astype(bf)                   # [28, 224]

    xq_all = np.ascontiguousarray(
        xr.reshape(n, 2, 128, h * w).transpose(0, 2, 1, 3)
        .reshape(n, 128, 2 * h * w)).astype(bf)
    xk_all = np.ascontiguousarray(
        xg.reshape(n, 2, 128, h * w).transpose(0, 2, 1, 3)
        .reshape(n, 128, 2 * h * w)).astype(bf)

    import os
    dbg = bool(os.environ.get("DBGTAPS"))
    nc = _get_program(dbg)
    core_ids = list(range(N_CORES))
    in_maps = []
    for i in core_ids:
        sl = slice(i * NS, (i + 1) * NS)
        in_maps.append({
            "xq": np.ascontiguousarray(xq_all[sl]),
            "xk": np.ascontiguousarray(xk_all[sl]),
            "wqt": wqt,
            "wvt": wvt,
            "at": at,
            "cvec": cvec,
        })

    res = run_bass_kernel_spmd(nc, in_maps, core_ids)
    global LAST_RESULTS
    LAST_RESULTS = res
    out = np.concatenate([r["y"] for r in res.results], axis=0)
    return out.astype(np.float32)


LAST_RESULTS = None


if __name__ == "__main__":
    # smoke test with random data
    rng = np.random.default_rng(0)
    inputs = {
        "x": rng.standard_normal((32, 256, 28, 28), dtype=np.float32),
        "lam": rng.random((1,), dtype=np.float32),
        "index": rng.integers(0, 32, (32,)),
        "scale_factor": 8,
        "Wq": (rng.standard_normal((128, 256)) * 0.01).astype(np.float32),
        "bq": np.zeros((128,), np.float32),
        "Wv": (rng.standard_normal((1, 257)) * 0.01).astype(np.float32),
        "bv": np.zeros((1,), np.float32),
    }
    y = kernel(**inputs)
    print("out", y.shape, y.dtype, float(y.min()), float(y.max()))


# revision 45
# speedup vs baseline: 1.0657x; 1.0033x over previous
"""PixelMixBlock V2 Trainium2 kernel.

Full inputs in, full output out. Data-parallel over the batch: 32 samples
-> 8 NeuronCores x 4 samples. Per sample (on device, all matmuls bf16):

  q = Wq' x          [128, 784]   (Wq' = Wq * 128**-0.25, folds the 1/sqrt(d))
  k = Wq' x_g        [128, 784]   (x_g = x[index], gathered on host)
  v = Wv[:256] x_g   [1, 784]
  sT[c] = k[:,c*112:+112].T @ q            [112, 784] PSUM   (7 chunks)
  E[c]  = exp(sT[c])                       bf16 SBUF
  [num;den] += [v_chunk, 1].T @ E[c]       [2, 784] PSUM accumulate
  m = num/den + (Wv[256]*(1-lam) + bv)     [1, 784]
  O = A @ m.reshape(28,28) @ A.T           (bilinear x8 upsample as matmuls)
  y[1] = 1/(1+exp(-O)); y[0] = 1 - y[1]

exp/sigmoid both use the ACT "exp" table set (sigmoid via exp + DVE
reciprocal) so no activation-table reload ever happens.
"""

import numpy as np
import ml_dtypes

import concourse.bass as bass
import concourse.bacc as bacc
import concourse.mybir as mybir
import concourse.tile as tile
from concourse.bass_utils import run_bass_kernel_spmd

BF16 = mybir.dt.bfloat16
F32 = mybir.dt.float32
AF = mybir.ActivationFunctionType
ALU = mybir.AluOpType

N_CORES = 8
NS = 4            # samples per core
CIN = 256         # input channels
INTER = 128       # Wq output channels
HW = 784          # 28*28
CH = 28
OUT = 224         # 28 * scale_factor(8)
CW = 112          # q-chunk width (784 = 7*112)
NCH = 7           # number of q chunks
NH = 392          # matmul N half (784 = 2*392)
OH = 112          # output row chunk (224 = 2*112)
W2W = 33          # nd lhsT width (v col + 31 zeros + ones col)


def _bilinear_matrix(n_in: int, n_out: int) -> np.ndarray:
    # Matches jax.image.resize(method='bilinear') for upscaling:
    # half-pixel sampling with edge clamp.
    scale = n_out / n_in
    A = np.zeros((n_out, n_in), np.float32)
    for i in range(n_out):
        src = (i + 0.5) / scale - 0.5
        j0 = int(np.floor(src))
        f = src - j0
        j0c = min(max(j0, 0), n_in - 1)
        j1c = min(max(j0 + 1, 0), n_in - 1)
        A[i, j0c] += 1.0 - f
        A[i, j1c] += f
    return A


def _build_program(dbg=False):
    nc = bacc.Bacc(
        "TRN2",
        target_bir_lowering=False,
        debug=False,
        enable_asserts=True,
        num_devices=N_CORES,
    )

    xq_d = nc.dram_tensor("xq", [NS, 128, 2 * HW], BF16, kind="ExternalInput").ap()
    xk_d = nc.dram_tensor("xk", [NS, 128, 2 * HW], BF16, kind="ExternalInput").ap()
    wqt_d = nc.dram_tensor("wqt", [128, 2 * INTER], BF16, kind="ExternalInput").ap()
    wvt_d = nc.dram_tensor("wvt", [128, 2], BF16, kind="ExternalInput").ap()
    at_d = nc.dram_tensor("at", [CH, OUT], BF16, kind="ExternalInput").ap()
    cvec_d = nc.dram_tensor("cvec", [CH, 1], F32, kind="ExternalInput").ap()
    y_d = nc.dram_tensor("y", [NS, 2, OUT, OUT], F32, kind="ExternalOutput").ap()
    if dbg:
        dq_d = nc.dram_tensor("dq", [NS, 128, HW], BF16, kind="ExternalOutput").ap()
        dk_d = nc.dram_tensor("dk", [NS, 128, HW], BF16, kind="ExternalOutput").ap()
        dv_d = nc.dram_tensor("dv", [NS, 1, HW], BF16, kind="ExternalOutput").ap()
        dw2_d = nc.dram_tensor("dw2", [NS, CW, 2 * NCH], BF16, kind="ExternalOutput").ap()
        de_d = nc.dram_tensor("de", [NS, CW, HW], BF16, kind="ExternalOutput").ap()
        dnd_d = nc.dram_tensor("dnd", [NS, 2, HW], F32, kind="ExternalOutput").ap()
        dmv_d = nc.dram_tensor("dmv", [NS, 1, HW], BF16, kind="ExternalOutput").ap()
        dmh_d = nc.dram_tensor("dmh", [NS, CH, CH], BF16, kind="ExternalOutput").ap()
        du_d = nc.dram_tensor("du", [NS, CH, OUT], BF16, kind="ExternalOutput").ap()

    with tile.TileContext(nc) as tc:
        with (
            tc.tile_pool(name="const", bufs=1) as cpool,
            tc.tile_pool(name="xin", bufs=NS) as xpool,
            tc.tile_pool(name="qk", bufs=2) as qkpool,
            tc.tile_pool(name="ew", bufs=6) as epool,
            tc.tile_pool(name="mk", bufs=2) as mkpool,
            tc.tile_pool(name="sg", bufs=3) as sgpool,
            tc.tile_pool(name="pqk", bufs=1, space="PSUM") as pqk,
            tc.tile_pool(name="pst", bufs=2, space="PSUM") as pst,
            tc.tile_pool(name="pnd", bufs=1, space="PSUM") as pnd,
        ):
            # --- input DMAs up front; sample-0 + weights first so the
            # first matmul starts as early as possible ---
            wqt_sb = cpool.tile([128, 2 * INTER], BF16, tag="wqt")
            wvt_sb = cpool.tile([128, 2], BF16, tag="wvt")
            at_sb = cpool.tile([CH, OUT], BF16, tag="at")
            cvec_sb = cpool.tile([CH, 1], F32, tag="cvec")
            one33 = cpool.tile([W2W, 1], F32, tag="one33")
            xq_sbs, xk_sbs = [], []
            for s in range(NS):
                xq_sb = xpool.tile([128, 2 * HW], BF16, tag="xq")
                xk_sb = xpool.tile([128, 2 * HW], BF16, tag="xk")
                xq_sbs.append(xq_sb)
                xk_sbs.append(xk_sb)
            nc.sync.dma_start(wqt_sb[:], wqt_d[:])
            # sample-0 inputs split by channel-chunk so the first matmuls can
            # start before the full tensors land
            nc.sync.dma_start(xq_sbs[0][:, 0:HW], xq_d[0][:, 0:HW])
            nc.gpsimd.dma_start(xk_sbs[0][:, 0:HW], xk_d[0][:, 0:HW])
            nc.sync.dma_start(xq_sbs[0][:, HW:2 * HW], xq_d[0][:, HW:2 * HW])
            nc.gpsimd.dma_start(xk_sbs[0][:, HW:2 * HW], xk_d[0][:, HW:2 * HW])
            nc.sync.dma_start(wvt_sb[:], wvt_d[:])
            for s in range(1, NS):
                nc.sync.dma_start(xq_sbs[s][:], xq_d[s])
                nc.sync.dma_start(xk_sbs[s][:], xk_d[s])
            nc.sync.dma_start(at_sb[:], at_d[:])
            nc.sync.dma_start(cvec_sb[:], cvec_d[:])

            # two persistent w2 tiles: [112, 33*7]; per chunk c the nd lhsT is
            # cols [33c, 33c+33): col 33c = v, cols +1..+31 = 0, col +32 = 1.
            # den then lands on PSUM partition 32 (32-aligned for DVE reads).
            nc.vector.memset(one33[:], 1.0)
            w2s = []
            for i in range(2):
                w2t = cpool.tile([CW, W2W * NCH], BF16, tag=f"w2_{i}")
                nc.vector.memset(w2t[:], 0.0)
                nc.vector.memset(w2t[:, 32:W2W * NCH:W2W], 1.0)
                w2s.append(w2t)

            def emit_head(s):
                """q/k/v matmuls + drains + w2 scatter for sample s."""
                xq_sb, xk_sb = xq_sbs[s], xk_sbs[s]
                q_sb = qkpool.tile([128, HW], BF16, tag="q")
                k_sb = qkpool.tile([128, HW], BF16, tag="k")
                for (src, dst) in ((xq_sb, q_sb), (xk_sb, k_sb)):
                    # [128, 2, 512] so each 392-wide half is PSUM-bank aligned
                    qps = pqk.tile([128, 2, 512], F32, tag="qk")
                    for c in range(2):
                        for h in range(2):
                            nc.tensor.matmul(
                                qps[:, h, 0:NH],
                                wqt_sb[:, c * INTER:(c + 1) * INTER],
                                src[:, c * HW + h * NH: c * HW + (h + 1) * NH],
                                start=(c == 0),
                                stop=(c == 1),
                            )
                    nc.vector.tensor_copy(
                        dst[:].rearrange("p (h f) -> p h f", h=2),
                        qps[:, :, 0:NH],
                    )

                # vT computed directly on PE: for q-chunk c,
                # vT[qq] = sum_i x_g[i, c*112+qq] * Wv[i]  (lhsT = x_g slice)
                vtp = pqk.tile([CW, NCH], F32, tag="qk")
                for c in range(NCH):
                    for ci in range(2):
                        nc.tensor.matmul(
                            vtp[:, c:c + 1],
                            xk_sb[:, ci * HW + c * CW: ci * HW + (c + 1) * CW],
                            wvt_sb[:, ci:ci + 1],
                            start=(ci == 0),
                            stop=(ci == 1),
                            skip_group_check=True,
                        )
                w2 = w2s[s % 2]
                nc.vector.tensor_copy(w2[:, 0:W2W * NCH:W2W], vtp[:])
                if dbg:
                    nc.sync.dma_start(dq_d[s], q_sb[:])
                    nc.sync.dma_start(dk_d[s], k_sb[:])
                return q_sb, k_sb, w2

            def emit_attention(s, q_sb, k_sb, w2):
                """sT -> exp -> num/den accumulate; returns PSUM nd tile."""
                ndps = pnd.tile([W2W, 2, 512], F32, tag="nd")

                def emit_st(c):
                    stps = pst.tile([CW, 2, 512], F32, tag="st")
                    for h in range(2):
                        nc.tensor.matmul(
                            stps[:, h, 0:NH],
                            k_sb[:, c * CW:(c + 1) * CW],
                            q_sb[:, h * NH:(h + 1) * NH],
                            start=True,
                            stop=True,
                        )
                    return stps

                # PE stream: sT(0), sT(1), nd(0), sT(2), nd(1), ... so the
                # next chunk's sT never queues behind an nd that waits on exp
                stps = emit_st(0)
                for c in range(NCH):
                    et = epool.tile([CW, HW], BF16, tag="e")
                    nc.scalar.activation(
                        et[:].rearrange("p (h f) -> p h f", h=2),
                        stps[:, :, 0:NH],
                        AF.Exp,
                    )
                    if dbg and c == 0:
                        nc.sync.dma_start(de_d[s], et[:])
                    if c + 1 < NCH:
                        stps = emit_st(c + 1)
                    for h in range(2):
                        nc.tensor.matmul(
                            ndps[:, h, 0:NH],
                            w2[:, W2W * c:W2W * (c + 1)],
                            et[:, h * NH:(h + 1) * NH],
                            start=(c == 0),
                            stop=(c == NCH - 1),
                            skip_group_check=True,
                        )
                return ndps

            def emit_tail(s, ndps):
                """mask reshape + divide, bilinear upsample, sigmoid, store."""
                nd_sb = mkpool.tile([W2W, HW], F32, tag="nd_sb")
                nc.vector.tensor_copy(
                    nd_sb[:].rearrange("p (h f) -> p h f", h=2),
                    ndps[:, :, 0:NH],
                )
                # reshape num/den rows to [28, 28] transposed layout with
                # 28 tiny PE transposes each (PE is idle here; avoids the
                # ~2.7us DMA latency in the tail chain)
                mtn_ps = pqk.tile([CH, CH], F32, tag="qk")
                for h in range(CH):
                    nc.tensor.transpose(
                        mtn_ps[:, h:h + 1],
                        nd_sb[0:1, CH * h:CH * (h + 1)],
                        one33[0:1, 0:1],
                    )
                mtd_ps = pnd.tile([CH, CH], F32, tag="nd")
                for h in range(CH):
                    nc.tensor.transpose(
                        mtd_ps[:, h:h + 1],
                        nd_sb[32:33, CH * h:CH * (h + 1)],
                        one33[32:33, 0:1],
                    )
                mhn = mkpool.tile([CH, CH], F32, tag="mhn")
                nc.vector.tensor_copy(mhn[:], mtn_ps[:])
                mhd = mkpool.tile([CH, CH], F32, tag="mhd")
                nc.vector.tensor_copy(mhd[:], mtd_ps[:])
                rcd = mkpool.tile([CH, CH], F32, tag="rcd")
                nc.vector.reciprocal(rcd[:], mhd[:])
                mtm = mkpool.tile([CH, CH], F32, tag="mtm")
                nc.vector.tensor_tensor(mtm[:], mhn[:], rcd[:], ALU.mult)
                # mh here is M^T: [w on partitions, h free]
                mh = mkpool.tile([CH, CH], BF16, tag="mh")
                nc.vector.tensor_scalar(
                    mh[:], mtm[:], cvec_sb[:, 0:1], None, ALU.add
                )
                if dbg:
                    nc.sync.dma_start(dnd_d[s, 0:1], nd_sb[0:1, :])
                    nc.sync.dma_start(dnd_d[s, 1:2], nd_sb[32:33, :])
                    nc.sync.dma_start(dmh_d[s], mh[:])

                ups = pqk.tile([CH, OUT], F32, tag="qk")
                nc.tensor.matmul(ups[:], mh[:], at_sb[:], start=True, stop=True)
                u_sb = mkpool.tile([CH, OUT], BF16, tag="u")
                nc.vector.tensor_copy(u_sb[:], ups[:])
                if dbg:
                    nc.sync.dma_start(du_d[s], u_sb[:])

                # both 112-row output chunks in one PSUM tile (bank-aligned
                # halves), one sigmoid-exp over both, fewer DVE ops
                ops = pqk.tile([OH, 2, 512], F32, tag="qk")
                for j in range(2):
                    nc.tensor.matmul(
                        ops[:, j, 0:OUT], at_sb[:, j * OH:(j + 1) * OH],
                        u_sb[:], start=True, stop=True,
                    )
                es = sgpool.tile([OH, 2, OUT], F32, tag="es")
                nc.scalar.activation(es[:], ops[:, :, 0:OUT], AF.Exp, scale=-1.0)
                t1 = sgpool.tile([OH, 2, OUT], F32, tag="t1")
                nc.vector.tensor_scalar(t1[:], es[:], 1.0, None, ALU.add)
                # sb4 layout: [112, (j, ch), 224]
                sb4 = sgpool.tile([OH, 4, OUT], F32, tag="sb4")
                nc.vector.reciprocal(
                    sb4[:].rearrange("p (j c) f -> p j c f", j=2)[:, :, 1, :],
                    t1[:],
                )
                nc.vector.tensor_scalar(
                    sb4[:].rearrange("p (j c) f -> p j c f", j=2)[:, :, 0, :],
                    sb4[:].rearrange("p (j c) f -> p j c f", j=2)[:, :, 1, :],
                    -1.0, 1.0, ALU.mult, ALU.add,
                )
                for j in range(2):
                    eng = nc.sync if j == 0 else nc.gpsimd
                    eng.dma_start(
                        y_d[s, :, j * OH:(j + 1) * OH, :]
                        .rearrange("c p f -> p c f"),
                        sb4[:, 2 * j:2 * j + 2, :],
                    )

            # software pipeline: emit head(s+1) before tail(s) so the next
            # sample's drains are not queued behind this sample's tail
            ctx = emit_head(0)
            for s in range(NS):
                ndps = emit_attention(s, *ctx)
                if s + 1 < NS:
                    ctx = emit_head(s + 1)
                emit_tail(s, ndps)

    nc.compile()
    return nc


_NC_CACHE = {}


def _get_program(dbg=False):
    if dbg not in _NC_CACHE:
        _NC_CACHE[dbg] = _build_program(dbg)
    return _NC_CACHE[dbg]


def kernel(x, lam, index, scale_factor, Wq, bq, Wv, bv):
    x = np.asarray(x, dtype=np.float32)
    lam = np.asarray(lam, dtype=np.float32)
    index = np.asarray(index).astype(np.int64)
    Wq = np.asarray(Wq, dtype=np.float32)
    Wv = np.asarray(Wv, dtype=np.float32)
    bv = np.asarray(bv, dtype=np.float32)

    n, C, h, w = x.shape
    bf = ml_dtypes.bfloat16

    xr = x.reshape(n, C, h * w)
    xg = xr[index]

    s4 = np.float32(INTER) ** np.float32(-0.25)
    WqT = np.ascontiguousarray((Wq * s4).T).astype(bf)          # [256, 128]
    wqt = np.ascontiguousarray(
        WqT.reshape(2, 128, INTER).transpose(1, 0, 2).reshape(128, 2 * INTER))
    wvt = np.ascontiguousarray(Wv[0, :C].astype(bf).reshape(2, 128).T)
    const = np.float32(Wv[0, C] * (1.0 - lam[0]) + bv[0])
    cvec = np.full((CH, 1), const, np.float32)
    A = _bilinear_matrix(CH, OUT)
    at = np.ascontiguousarray(A.T).astype(bf)                   # [28, 224]

    xq_all = np.ascontiguousarray(
        xr.reshape(n, 2, 128, h * w).transpose(0, 2, 1, 3)
        .reshape(n, 128, 2 * h * w)).astype(bf)
    xk_all = np.ascontiguousarray(
        xg.reshape(n, 2, 128, h * w).transpose(0, 2, 1, 3)
        .reshape(n, 128, 2 * h * w)).astype(bf)

    import os
    dbg = bool(os.environ.get("DBGTAPS"))
    nc = _get_program(dbg)
    core_ids = list(range(N_CORES))
    in_maps = []
    for i in core_ids:
        sl = slice(i * NS, (i + 1) * NS)
        in_maps.append({
            "xq": np.ascontiguousarray(xq_all[sl]),
            "xk": np.ascontiguousarray(xk_all[sl]),
            "wqt": wqt,
            "wvt": wvt,
            "at": at,
            "cvec": cvec,
        })

    res = run_bass_kernel_spmd(nc, in_maps, core_ids)
    global LAST_RESULTS
    LAST_RESULTS = res
    out = np.concatenate([r["y"] for r in res.results], axis=0)
    return out.astype(np.float32)


LAST_RESULTS = None


if __name__ == "__main__":
    # smoke test with random data
    rng = np.random.default_rng(0)
    inputs = {
        "x": rng.standard_normal((32, 256, 28, 28), dtype=np.float32),
        "lam": rng.random((1,), dtype=np.float32),
        "index": rng.integers(0, 32, (32,)),
        "scale_factor": 8,
        "Wq": (rng.standard_normal((128, 256)) * 0.01).astype(np.float32),
        "bq": np.zeros((128,), np.float32),
        "Wv": (rng.standard_normal((1, 257)) * 0.01).astype(np.float32),
        "bv": np.zeros((1,), np.float32),
    }
    y = kernel(**inputs)
    print("out", y.shape, y.dtype, float(y.min()), float(y.max()))
